# revision 13
# baseline (speedup 1.0000x reference)
"""DeepAR (2-layer LSTM encoder + LSTM-cell decoder) Trainium2 Bass kernel.

Sharding: pure data parallel, batch 1024 -> 128 per core across 8 cores
(batch 128 == SBUF partition width).

Per-core design:
  - gates in [128 batch, 2048 gate] layout, gate order reordered to
    [g, i, f, o]: tanh(g) finishes first so the DVE chain starts early,
    and one sigmoid covers cols 512:2048.
  - all matmuls bf16 (1 cyc/col on PE), fp32 PSUM accumulation; biases ride
    ones-rows (K=1 matmuls) in bf16. (float32r is avoided: it silently
    corrupts results on HW in this setup.)
  - n-outer matmul emission: each 512-col PSUM bank completes early so
    ACT can start before the whole gate tensor is done.
  - recurrent h produced in bf16, transposed to stationary [K,M] layout by
    the DMA xbar (no PE/PSUM cost); layer-0 and layer-1 transposes go to
    different HWDGE queues (sync vs scalar) to halve queue serialization.
  - layer 1 runs one step behind layer 0 so the PE always has independent
    matmul work while layer 0's elementwise chain runs (keeps HAM warm).
  - decoder context contribution precomputed once and injected into PSUM
    via identity matmul each step; mu/sigma heads are DVE dot-products
    (scalar_tensor_tensor with accum_out), off the critical path.
"""
import numpy as np
import ml_dtypes

import concourse.bass as bass
import concourse.mybir as mybir
import concourse.tile as tile
from concourse import bacc
from concourse.bass_utils import run_bass_kernel_spmd
from concourse.masks import make_identity

F32 = mybir.dt.float32
BF16 = mybir.dt.bfloat16
AF = mybir.ActivationFunctionType
ALU = mybir.AluOpType

B, T_ENC, H_DEC = 1024, 168, 24
ENC_IN, DEC_IN, HID = 32, 16, 512
G = 4 * HID  # 2048
NCORES = 8
BL = B // NCORES  # 128 batch per core
XCHUNK = 28  # encoder-input steps per DMA chunk

# gate reorder: torch order [i, f, g, o] -> [g, i, f, o]
_PERM = np.concatenate([np.arange(1024, 1536), np.arange(0, 512),
                        np.arange(512, 1024), np.arange(1536, 2048)])


def _bf16(x):
    return np.ascontiguousarray(x.astype(ml_dtypes.bfloat16))


def _f32(x):
    return np.ascontiguousarray(x.astype(np.float32))


def _wT_kxn(W):
    """[4H, D] gate-major weight -> reordered W.T as [128, D//128, 4H] bf16."""
    Wt = W[_PERM].T  # [D, 2048]
    D = Wt.shape[0]
    return _bf16(Wt.reshape(D // 128, 128, G).transpose(1, 0, 2))


def build_kernel(T=T_ENC, HD=H_DEC):
    nc = bacc.Bacc("TRN2", target_bir_lowering=False, debug=False,
                   num_devices=NCORES)

    def din(name, shape, dt):
        return nc.dram_tensor(name, shape, dt, kind="ExternalInput").ap()

    x_d = din("x", [ENC_IN + 1, T, BL], BF16)        # enc features + ones row
    w0_d = din("w0", [ENC_IN + 1, G], BF16)           # W_ih0T + bias row
    wh0_d = din("wh0", [128, 4, G], BF16)
    wi1_d = din("wi1", [128, 4, G], BF16)
    wh1_d = din("wh1", [128, 4, G], BF16)
    wctx_d = din("wctx", [128, 4, G], BF16)
    whd_d = din("whd", [128, 4, G], BF16)
    be_d = din("be", [33, G + 128], BF16)  # row0: b1|ones, row32: bd|ones
    covy_d = din("covy", [DEC_IN + 1, HD, BL], BF16)  # dec covariates + y_prev
    wcy_d = din("wcy", [DEC_IN + 1, G], BF16)
    # head weights broadcast across partitions + per-partition biases:
    # cols 0:512 W_mu, 512:1024 W_sig, 1024 b_mu, 1025 b_sig
    wms_d = din("wms", [128, 2 * HID + 2], F32)

    mu_d = nc.dram_tensor("mu", [BL, HD], F32, kind="ExternalOutput").ap()
    sg_d = nc.dram_tensor("sg", [BL, HD], F32, kind="ExternalOutput").ap()

    with tile.TileContext(nc) as tc:
        _emit(tc, T, HD, x_d, w0_d, wh0_d, wi1_d, wh1_d, wctx_d, whd_d,
              be_d, covy_d, wcy_d, wms_d, mu_d, sg_d)
    nc.compile()
    return nc


def _emit(tc, T, HD, x_d, w0_d, wh0_d, wi1_d, wh1_d, wctx_d, whd_d,
          be_d, covy_d, wcy_d, wms_d, mu_d, sg_d):
    nc = tc.nc
    mm = nc.tensor.matmul

    with (
        tc.tile_pool(name="const", bufs=1) as cp,
        tc.tile_pool(name="xp", bufs=2) as xp,
        tc.tile_pool(name="sig", bufs=3) as sigp,
        tc.tile_pool(name="small", bufs=3) as smp,
        tc.tile_pool(name="hp", bufs=2) as hp,
        tc.tile_pool(name="htp", bufs=3) as htp,
        tc.tile_pool(name="psum", bufs=2, space="PSUM") as pp,
    ):
        # ---- persistent tiles / weight loads ----
        def load(name, dram, shape, dt):
            t = cp.tile(shape, dt, tag=name)
            nc.sync.dma_start(t[:], dram[:])
            return t

        w0 = load("w0", w0_d, [ENC_IN + 1, G], BF16)
        wh0 = load("wh0", wh0_d, [128, 4, G], BF16)
        wi1 = load("wi1", wi1_d, [128, 4, G], BF16)
        wh1 = load("wh1", wh1_d, [128, 4, G], BF16)
        wctx = load("wctx", wctx_d, [128, 4, G], BF16)
        whd = load("whd", whd_d, [128, 4, G], BF16)
        be = load("be", be_d, [33, G + 128], BF16)
        covy = load("covy", covy_d, [DEC_IN + 1, HD, BL], BF16)
        wcy = load("wcy", wcy_d, [DEC_IN + 1, G], BF16)
        wms = load("wms", wms_d, [128, 2 * HID + 2], F32)

        ident = cp.tile([128, 128], BF16, tag="ident")
        make_identity(nc, ident[:])

        ones_r = be[0:1, G:G + 128]
        ones32_r = be[32:33, G:G + 128]
        b1_r = be[0:1, 0:G]
        bd_r = be[32:33, 0:G]

        c0 = cp.tile([128, HID], F32, tag="c0")
        c1 = cp.tile([128, HID], F32, tag="c1")
        cd = cp.tile([128, HID], F32, tag="cd")
        mu_b = cp.tile([128, HD], F32, tag="mu_b")
        sp_b = cp.tile([128, HD], F32, tag="sp_b")
        sg_b = cp.tile([128, HD], F32, tag="sg_b")

        NS = G // 512  # 4 n-chunks

        def cell(g, c, first, h_tag):
            """gates psum [g|i|f|o] -> h (bf16 [128, HID]) via ACT/DVE.

            ACT order: tanh(g), sigmoid(i) first so DVE starts early.
            """
            tg = smp.tile([128, HID], F32, tag="tg")
            nc.scalar.activation(tg[:], g[:, 0:HID], AF.Tanh)
            si = smp.tile([128, HID], F32, tag="si")
            nc.scalar.activation(si[:], g[:, HID:2 * HID], AF.Sigmoid)
            sfo = sigp.tile([128, 2 * HID], F32, tag="sfo")
            nc.scalar.activation(sfo[:], g[:, 2 * HID:G], AF.Sigmoid)
            if first:
                nc.vector.tensor_mul(c[:], si[:], tg[:])
            else:
                m1 = smp.tile([128, HID], F32, tag="m1")
                nc.vector.tensor_mul(m1[:], si[:], tg[:])
                m2 = smp.tile([128, HID], F32, tag="m2")
                nc.vector.tensor_mul(m2[:], sfo[:, 0:HID], c[:])
                nc.vector.tensor_add(c[:], m1[:], m2[:])
            tcn = smp.tile([128, HID], F32, tag="tc")
            nc.scalar.activation(tcn[:], c[:], AF.Tanh)
            h = hp.tile([128, HID], BF16, tag=h_tag)
            nc.vector.tensor_mul(h[:], sfo[:, HID:2 * HID], tcn[:])
            return h

        def transp(h, tag, eng):
            ht = htp.tile([128, 4, 128], BF16, tag=tag)
            for k in range(4):
                eng.dma_start(ht[:, k, :], h[:, k * 128:(k + 1) * 128],
                              transpose=True)
            return ht

        # ================= encoder =================
        # L1 runs one step behind L0: while L0(t)'s elementwise chain runs
        # on ACT/DVE/DMA, the PE stays busy on L1(t-1)'s matmuls.
        h0T_hist = {}
        h1T = None
        x_sb = None

        def layer1(t):
            nonlocal h1T
            g1 = pp.tile([128, G], F32, tag="g")
            for n in range(NS):
                s = slice(n * 512, (n + 1) * 512)
                mm(g1[:, s], ones_r, b1_r[:, s], start=True, stop=False)
                for k in range(4):
                    mm(g1[:, s], h0T_hist[t][:, k, :], wi1[:, k, s],
                       start=False, stop=(t == 0 and k == 3))
                if t > 0:
                    for k in range(4):
                        mm(g1[:, s], h1T[:, k, :], wh1[:, k, s],
                           start=False, stop=(k == 3))
            h1 = cell(g1, c1, t == 0, "h1")
            h1T = transp(h1, "h1T", nc.scalar)

        for t in range(T):
            if t % XCHUNK == 0:
                nx = min(XCHUNK, T - t)
                x_sb = xp.tile([ENC_IN + 1, XCHUNK, BL], BF16, tag="x")
                nc.scalar.dma_start(x_sb[:, :nx, :], x_d[:, t:t + nx, :])
            ti = t % XCHUNK
            # ---- layer 0, step t ----
            g0 = pp.tile([128, G], F32, tag="g")
            for n in range(NS):
                s = slice(n * 512, (n + 1) * 512)
                mm(g0[:, s], x_sb[:, ti, :], w0[:, s],
                   start=True, stop=(t == 0))
                if t > 0:
                    for k in range(4):
                        mm(g0[:, s], h0T_hist[t - 1][:, k, :], wh0[:, k, s],
                           start=False, stop=(k == 3))
            h0 = cell(g0, c0, t == 0, "h0")
            h0T_hist[t] = transp(h0, "h0T", nc.sync)
            h0T_hist.pop(t - 2, None)
            # ---- layer 1, step t-1 ----
            if t >= 1:
                layer1(t - 1)
        layer1(T - 1)

        # ================= decoder =================
        # one-time: ctx_pre = context @ W_ctx.T + (b_ihd + b_hhd)
        cps = pp.tile([128, G], F32, tag="g")
        for n in range(NS):
            s = slice(n * 512, (n + 1) * 512)
            mm(cps[:, s], ones32_r, bd_r[:, s], start=True, stop=False)
            for k in range(4):
                mm(cps[:, s], h1T[:, k, :], wctx[:, k, s],
                   start=False, stop=(k == 3))
        ctxp = cp.tile([128, G], BF16, tag="ctxp")
        nc.scalar.copy(ctxp[:], cps[:])

        hdT = None
        for t in range(HD):
            gd = pp.tile([128, G], F32, tag="g")
            for n in range(NS):
                s = slice(n * 512, (n + 1) * 512)
                mm(gd[:, s], ident[:], ctxp[:, s], start=True, stop=False)
                mm(gd[:, s], covy[:, t, :], wcy[:, s],
                   start=False, stop=(t == 0))
                if t > 0:
                    for k in range(4):
                        mm(gd[:, s], hdT[:, k, :], whd[:, k, s],
                           start=False, stop=(k == 3))
            hd = cell(gd, cd, t == 0, "hd")
            hdT = transp(hd, "hdT", nc.sync)

            # heads: mu/sigma dot-products on DVE, off the critical path
            hsc = smp.tile([128, HID], F32, tag="hsc")
            nc.vector.scalar_tensor_tensor(
                hsc[:], hd[:], 1.0, wms[:, 0:HID],
                op0=ALU.mult, op1=ALU.mult, accum_out=mu_b[:, t:t + 1])
            hsc2 = smp.tile([128, HID], F32, tag="hsc2")
            nc.vector.scalar_tensor_tensor(
                hsc2[:], hd[:], 1.0, wms[:, HID:2 * HID],
                op0=ALU.mult, op1=ALU.mult, accum_out=sp_b[:, t:t + 1])

        # add head biases; sigma = softplus(x) + 1e-6 via ln(exp(x)+1)
        nc.vector.tensor_scalar_add(mu_b[:], mu_b[:],
                                    wms[:, 2 * HID:2 * HID + 1])
        nc.vector.tensor_scalar_add(sp_b[:], sp_b[:],
                                    wms[:, 2 * HID + 1:2 * HID + 2])
        nc.scalar.activation(sp_b[:], sp_b[:], AF.Exp)
        nc.scalar.activation(sg_b[:], sp_b[:], AF.Ln, bias=1.0)
        nc.vector.tensor_scalar_add(sg_b[:], sg_b[:], 1e-6)
        nc.sync.dma_start(mu_d[:], mu_b[:])
        nc.sync.dma_start(sg_d[:], sg_b[:])


def _make_be(b1, bdv):
    be = np.zeros((33, G + 128), np.float32)
    be[0, :G] = b1
    be[32, :G] = bdv
    be[0, G:] = 1.0
    be[32, G:] = 1.0
    return _bf16(be)


def _make_wms(W_mu, W_sig, b_mu, b_sig):
    w = np.zeros((128, 2 * HID + 2), np.float32)
    w[:, 0:HID] = W_mu[0][None, :]
    w[:, HID:2 * HID] = W_sig[0][None, :]
    w[:, 2 * HID] = b_mu[0]
    w[:, 2 * HID + 1] = b_sig[0]
    return _f32(w)


def prep_inputs(inputs, T=T_ENC, HD=H_DEC):
    """Full-batch inputs -> list of per-core input maps (host layout prep)."""
    enc = _f32(np.asarray(inputs["enc_inp"]))[:, :T]
    dec = _f32(np.asarray(inputs["dec_inp"]))[:, :HD]
    tgt = _f32(np.asarray(inputs["tgt"]))[:, :HD]

    W_ih0, W_hh0 = np.asarray(inputs["W_ih0"]), np.asarray(inputs["W_hh0"])
    W_ih1, W_hh1 = np.asarray(inputs["W_ih1"]), np.asarray(inputs["W_hh1"])
    W_ihd, W_hhd = np.asarray(inputs["W_ihd"]), np.asarray(inputs["W_hhd"])
    b0 = _f32(np.asarray(inputs["b_ih0"]) + np.asarray(inputs["b_hh0"]))[_PERM]
    b1 = _f32(np.asarray(inputs["b_ih1"]) + np.asarray(inputs["b_hh1"]))[_PERM]
    bdv = _f32(np.asarray(inputs["b_ihd"]) + np.asarray(inputs["b_hhd"]))[_PERM]
    W_mu, b_mu = np.asarray(inputs["W_mu"]), np.asarray(inputs["b_mu"])
    W_sig, b_sig = np.asarray(inputs["W_sig"]), np.asarray(inputs["b_sig"])

    w0 = np.concatenate([W_ih0[_PERM].T, b0[None, :]], 0)  # [33, 2048]
    shared = {
        "w0": _bf16(w0),
        "wh0": _wT_kxn(W_hh0),
        "wi1": _wT_kxn(W_ih1),
        "wh1": _wT_kxn(W_hh1),
        "wctx": _wT_kxn(W_ihd[:, DEC_IN:DEC_IN + HID]),
        "whd": _wT_kxn(W_hhd),
        "be": _make_be(b1, bdv),
        "wcy": _bf16(np.concatenate(
            [W_ihd[_PERM][:, :DEC_IN].T, W_ihd[_PERM][:, DEC_IN + HID:].T], 0)),
        "wms": _make_wms(W_mu, W_sig, b_mu, b_sig),
    }

    in_maps = []
    for c in range(NCORES):
        sl = slice(c * BL, (c + 1) * BL)
        xe = np.ones((ENC_IN + 1, T, BL), np.float32)
        xe[:ENC_IN] = enc[sl].transpose(2, 1, 0)
        cy = np.zeros((DEC_IN + 1, HD, BL), np.float32)
        cy[:DEC_IN] = dec[sl].transpose(2, 1, 0)
        cy[DEC_IN, 1:] = tgt[sl, :HD - 1].T
        m = dict(shared)
        m["x"] = _bf16(xe)
        m["covy"] = _bf16(cy)
        in_maps.append(m)
    return in_maps


_NC_CACHE = {}


def _get_nc(T=T_ENC, HD=H_DEC):
    key = (T, HD)
    if key not in _NC_CACHE:
        _NC_CACHE[key] = build_kernel(T, HD)
    return _NC_CACHE[key]


def run(inputs, T=T_ENC, HD=H_DEC, **kw):
    nc = _get_nc(T, HD)
    in_maps = prep_inputs(inputs, T, HD)
    res = run_bass_kernel_spmd(nc, in_maps, core_ids=list(range(NCORES)), **kw)
    mu = np.concatenate([res.results[c]["mu"] for c in range(NCORES)], 0)
    sg = np.concatenate([res.results[c]["sg"] for c in range(NCORES)], 0)
    return (mu, sg), res


def kernel(**inputs):
    (mu, sg), _ = run(inputs)
    return mu, sg


# revision 14
# speedup vs baseline: 1.0608x; 1.0608x over previous
"""DeepAR (2-layer LSTM encoder + LSTM-cell decoder) Trainium2 Bass kernel.

Sharding: pure data parallel, batch 1024 -> 128 per core across 8 cores
(batch 128 == SBUF partition width).

Per-core design:
  - gates in [128 batch, 2048 gate] layout, gate order reordered to
    [g, i, f, o]: tanh(g) finishes first so the DVE chain starts early,
    and one sigmoid covers cols 512:2048.
  - all matmuls bf16 (1 cyc/col on PE), fp32 PSUM accumulation; biases ride
    ones-rows (K=1 matmuls) in bf16. (float32r is avoided: it silently
    corrupts results on HW in this setup.)
  - n-outer matmul emission: each 512-col PSUM bank completes early so
    ACT can start before the whole gate tensor is done.
  - recurrent h produced in bf16, transposed to stationary [K,M] layout by
    the DMA xbar (no PE/PSUM cost); layer-0 and layer-1 transposes go to
    different HWDGE queues (sync vs scalar) to halve queue serialization.
  - layer 1 runs one step behind layer 0 so the PE always has independent
    matmul work while layer 0's elementwise chain runs (keeps HAM warm).
  - decoder context contribution precomputed once and injected into PSUM
    via identity matmul each step; mu/sigma heads are DVE dot-products
    (scalar_tensor_tensor with accum_out), off the critical path.
"""
import numpy as np
import ml_dtypes

import concourse.bass as bass
import concourse.mybir as mybir
import concourse.tile as tile
from concourse import bacc
from concourse.bass_utils import run_bass_kernel_spmd
from concourse.masks import make_identity

F32 = mybir.dt.float32
BF16 = mybir.dt.bfloat16
AF = mybir.ActivationFunctionType
ALU = mybir.AluOpType

B, T_ENC, H_DEC = 1024, 168, 24
ENC_IN, DEC_IN, HID = 32, 16, 512
G = 4 * HID  # 2048
NCORES = 8
BL = B // NCORES  # 128 batch per core
XCHUNK = 28  # encoder-input steps per DMA chunk

# gate reorder: torch order [i, f, g, o] -> [g, i, f, o]
_PERM = np.concatenate([np.arange(1024, 1536), np.arange(0, 512),
                        np.arange(512, 1024), np.arange(1536, 2048)])


def _bf16(x):
    return np.ascontiguousarray(x.astype(ml_dtypes.bfloat16))


def _f32(x):
    return np.ascontiguousarray(x.astype(np.float32))


def _wT_kxn(W):
    """[4H, D] gate-major weight -> reordered W.T as [128, D//128, 4H] bf16."""
    Wt = W[_PERM].T  # [D, 2048]
    D = Wt.shape[0]
    return _bf16(Wt.reshape(D // 128, 128, G).transpose(1, 0, 2))


def build_kernel(T=T_ENC, HD=H_DEC):
    nc = bacc.Bacc("TRN2", target_bir_lowering=False, debug=False,
                   num_devices=NCORES)

    def din(name, shape, dt):
        return nc.dram_tensor(name, shape, dt, kind="ExternalInput").ap()

    x_d = din("x", [ENC_IN + 1, T, BL], BF16)        # enc features + ones row
    w0_d = din("w0", [ENC_IN + 1, G], BF16)           # W_ih0T + bias row
    wh0_d = din("wh0", [128, 4, G], BF16)
    wi1_d = din("wi1", [128, 4, G], BF16)
    wh1_d = din("wh1", [128, 4, G], BF16)
    wctx_d = din("wctx", [128, 4, G], BF16)
    whd_d = din("whd", [128, 4, G], BF16)
    be_d = din("be", [33, G + 128], BF16)  # row0: b1|ones, row32: bd|ones
    covy_d = din("covy", [DEC_IN + 1, HD, BL], BF16)  # dec covariates + y_prev
    wcy_d = din("wcy", [DEC_IN + 1, G], BF16)
    # head weights broadcast across partitions + per-partition biases:
    # cols 0:512 W_mu, 512:1024 W_sig, 1024 b_mu, 1025 b_sig
    wms_d = din("wms", [128, 2 * HID + 2], F32)

    mu_d = nc.dram_tensor("mu", [BL, HD], F32, kind="ExternalOutput").ap()
    sg_d = nc.dram_tensor("sg", [BL, HD], F32, kind="ExternalOutput").ap()

    with tile.TileContext(nc) as tc:
        _emit(tc, T, HD, x_d, w0_d, wh0_d, wi1_d, wh1_d, wctx_d, whd_d,
              be_d, covy_d, wcy_d, wms_d, mu_d, sg_d)
    nc.compile()
    return nc


def _emit(tc, T, HD, x_d, w0_d, wh0_d, wi1_d, wh1_d, wctx_d, whd_d,
          be_d, covy_d, wcy_d, wms_d, mu_d, sg_d):
    nc = tc.nc
    mm = nc.tensor.matmul

    with (
        tc.tile_pool(name="const", bufs=1) as cp,
        tc.tile_pool(name="xp", bufs=2) as xp,
        tc.tile_pool(name="sig", bufs=3) as sigp,
        tc.tile_pool(name="small", bufs=3) as smp,
        tc.tile_pool(name="hp", bufs=2) as hp,
        tc.tile_pool(name="htp", bufs=3) as htp,
        tc.tile_pool(name="psum", bufs=2, space="PSUM") as pp,
    ):
        # ---- persistent tiles / weight loads ----
        def load(name, dram, shape, dt):
            t = cp.tile(shape, dt, tag=name)
            nc.sync.dma_start(t[:], dram[:])
            return t

        w0 = load("w0", w0_d, [ENC_IN + 1, G], BF16)
        wh0 = load("wh0", wh0_d, [128, 4, G], BF16)
        wi1 = load("wi1", wi1_d, [128, 4, G], BF16)
        wh1 = load("wh1", wh1_d, [128, 4, G], BF16)
        wctx = load("wctx", wctx_d, [128, 4, G], BF16)
        whd = load("whd", whd_d, [128, 4, G], BF16)
        be = load("be", be_d, [33, G + 128], BF16)
        covy = load("covy", covy_d, [DEC_IN + 1, HD, BL], BF16)
        wcy = load("wcy", wcy_d, [DEC_IN + 1, G], BF16)
        wms = load("wms", wms_d, [128, 2 * HID + 2], F32)

        ident = cp.tile([128, 128], BF16, tag="ident")
        make_identity(nc, ident[:])

        ones_r = be[0:1, G:G + 128]
        ones32_r = be[32:33, G:G + 128]
        b1_r = be[0:1, 0:G]
        bd_r = be[32:33, 0:G]

        c0 = cp.tile([128, HID], F32, tag="c0")
        c1 = cp.tile([128, HID], F32, tag="c1")
        cd = cp.tile([128, HID], F32, tag="cd")
        mu_b = cp.tile([128, HD], F32, tag="mu_b")
        sp_b = cp.tile([128, HD], F32, tag="sp_b")
        sg_b = cp.tile([128, HD], F32, tag="sg_b")

        NS = G // 512  # 4 n-chunks

        def cell(g, c, first, h_tag):
            """gates psum [g|i|f|o] -> h (bf16 [128, HID]) via ACT/DVE.

            ACT order: tanh(g), sigmoid(i) first so DVE starts early.
            """
            tg = smp.tile([128, HID], F32, tag="tg")
            nc.scalar.activation(tg[:], g[:, 0:HID], AF.Tanh)
            si = smp.tile([128, HID], F32, tag="si")
            nc.scalar.activation(si[:], g[:, HID:2 * HID], AF.Sigmoid)
            sfo = sigp.tile([128, 2 * HID], F32, tag="sfo")
            nc.scalar.activation(sfo[:], g[:, 2 * HID:G], AF.Sigmoid)
            if first:
                nc.vector.tensor_mul(c[:], si[:], tg[:])
            else:
                m1 = smp.tile([128, HID], F32, tag="m1")
                nc.vector.tensor_mul(m1[:], si[:], tg[:])
                m2 = smp.tile([128, HID], F32, tag="m2")
                nc.vector.tensor_mul(m2[:], sfo[:, 0:HID], c[:])
                nc.vector.tensor_add(c[:], m1[:], m2[:])
            tcn = smp.tile([128, HID], F32, tag="tc")
            nc.scalar.activation(tcn[:], c[:], AF.Tanh)
            h = hp.tile([128, HID], BF16, tag=h_tag)
            nc.vector.tensor_mul(h[:], sfo[:, HID:2 * HID], tcn[:])
            return h

        def transp(h, tag, eng):
            ht = htp.tile([128, 4, 128], BF16, tag=tag)
            for k in range(4):
                eng.dma_start(ht[:, k, :], h[:, k * 128:(k + 1) * 128],
                              transpose=True)
            return ht

        # ================= encoder =================
        # L1 runs one step behind L0: while L0(t)'s elementwise chain runs
        # on ACT/DVE/DMA, the PE stays busy on L1(t-1)'s matmuls.
        h0T_hist = {}
        h1T = None
        x_sb = None

        def layer1(t):
            nonlocal h1T
            g1 = pp.tile([128, G], F32, tag="g")
            for n in range(NS):
                s = slice(n * 512, (n + 1) * 512)
                mm(g1[:, s], ones_r, b1_r[:, s], start=True, stop=False)
                for k in range(4):
                    mm(g1[:, s], h0T_hist[t][:, k, :], wi1[:, k, s],
                       start=False, stop=(t == 0 and k == 3))
                if t > 0:
                    for k in range(4):
                        mm(g1[:, s], h1T[:, k, :], wh1[:, k, s],
                           start=False, stop=(k == 3))
            h1 = cell(g1, c1, t == 0, "h1")
            h1T = transp(h1, "h1T", nc.sync)

        for t in range(T):
            if t % XCHUNK == 0:
                nx = min(XCHUNK, T - t)
                x_sb = xp.tile([ENC_IN + 1, XCHUNK, BL], BF16, tag="x")
                nc.sync.dma_start(x_sb[:, :nx, :], x_d[:, t:t + nx, :])
            ti = t % XCHUNK
            # ---- layer 0, step t ----
            g0 = pp.tile([128, G], F32, tag="g")
            for n in range(NS):
                s = slice(n * 512, (n + 1) * 512)
                mm(g0[:, s], x_sb[:, ti, :], w0[:, s],
                   start=True, stop=(t == 0))
                if t > 0:
                    for k in range(4):
                        mm(g0[:, s], h0T_hist[t - 1][:, k, :], wh0[:, k, s],
                           start=False, stop=(k == 3))
            h0 = cell(g0, c0, t == 0, "h0")
            h0T_hist[t] = transp(h0, "h0T", nc.sync)
            h0T_hist.pop(t - 2, None)
            # ---- layer 1, step t-1 ----
            if t >= 1:
                layer1(t - 1)
        layer1(T - 1)

        # ================= decoder =================
        # one-time: ctx_pre = context @ W_ctx.T + (b_ihd + b_hhd)
        cps = pp.tile([128, G], F32, tag="g")
        for n in range(NS):
            s = slice(n * 512, (n + 1) * 512)
            mm(cps[:, s], ones32_r, bd_r[:, s], start=True, stop=False)
            for k in range(4):
                mm(cps[:, s], h1T[:, k, :], wctx[:, k, s],
                   start=False, stop=(k == 3))
        ctxp = cp.tile([128, G], BF16, tag="ctxp")
        nc.scalar.copy(ctxp[:], cps[:])

        hdT = None
        for t in range(HD):
            gd = pp.tile([128, G], F32, tag="g")
            for n in range(NS):
                s = slice(n * 512, (n + 1) * 512)
                mm(gd[:, s], ident[:], ctxp[:, s], start=True, stop=False)
                mm(gd[:, s], covy[:, t, :], wcy[:, s],
                   start=False, stop=(t == 0))
                if t > 0:
                    for k in range(4):
                        mm(gd[:, s], hdT[:, k, :], whd[:, k, s],
                           start=False, stop=(k == 3))
            hd = cell(gd, cd, t == 0, "hd")
            hdT = transp(hd, "hdT", nc.sync)

            # heads: mu/sigma dot-products on DVE, off the critical path
            hsc = smp.tile([128, HID], F32, tag="hsc")
            nc.vector.scalar_tensor_tensor(
                hsc[:], hd[:], 1.0, wms[:, 0:HID],
                op0=ALU.mult, op1=ALU.mult, accum_out=mu_b[:, t:t + 1])
            hsc2 = smp.tile([128, HID], F32, tag="hsc2")
            nc.vector.scalar_tensor_tensor(
                hsc2[:], hd[:], 1.0, wms[:, HID:2 * HID],
                op0=ALU.mult, op1=ALU.mult, accum_out=sp_b[:, t:t + 1])

        # add head biases; sigma = softplus(x) + 1e-6 via ln(exp(x)+1)
        nc.vector.tensor_scalar_add(mu_b[:], mu_b[:],
                                    wms[:, 2 * HID:2 * HID + 1])
        nc.vector.tensor_scalar_add(sp_b[:], sp_b[:],
                                    wms[:, 2 * HID + 1:2 * HID + 2])
        nc.scalar.activation(sp_b[:], sp_b[:], AF.Exp)
        nc.scalar.activation(sg_b[:], sp_b[:], AF.Ln, bias=1.0)
        nc.vector.tensor_scalar_add(sg_b[:], sg_b[:], 1e-6)
        nc.sync.dma_start(mu_d[:], mu_b[:])
        nc.sync.dma_start(sg_d[:], sg_b[:])


def _make_be(b1, bdv):
    be = np.zeros((33, G + 128), np.float32)
    be[0, :G] = b1
    be[32, :G] = bdv
    be[0, G:] = 1.0
    be[32, G:] = 1.0
    return _bf16(be)


def _make_wms(W_mu, W_sig, b_mu, b_sig):
    w = np.zeros((128, 2 * HID + 2), np.float32)
    w[:, 0:HID] = W_mu[0][None, :]
    w[:, HID:2 * HID] = W_sig[0][None, :]
    w[:, 2 * HID] = b_mu[0]
    w[:, 2 * HID + 1] = b_sig[0]
    return _f32(w)


def prep_inputs(inputs, T=T_ENC, HD=H_DEC):
    """Full-batch inputs -> list of per-core input maps (host layout prep)."""
    enc = _f32(np.asarray(inputs["enc_inp"]))[:, :T]
    dec = _f32(np.asarray(inputs["dec_inp"]))[:, :HD]
    tgt = _f32(np.asarray(inputs["tgt"]))[:, :HD]

    W_ih0, W_hh0 = np.asarray(inputs["W_ih0"]), np.asarray(inputs["W_hh0"])
    W_ih1, W_hh1 = np.asarray(inputs["W_ih1"]), np.asarray(inputs["W_hh1"])
    W_ihd, W_hhd = np.asarray(inputs["W_ihd"]), np.asarray(inputs["W_hhd"])
    b0 = _f32(np.asarray(inputs["b_ih0"]) + np.asarray(inputs["b_hh0"]))[_PERM]
    b1 = _f32(np.asarray(inputs["b_ih1"]) + np.asarray(inputs["b_hh1"]))[_PERM]
    bdv = _f32(np.asarray(inputs["b_ihd"]) + np.asarray(inputs["b_hhd"]))[_PERM]
    W_mu, b_mu = np.asarray(inputs["W_mu"]), np.asarray(inputs["b_mu"])
    W_sig, b_sig = np.asarray(inputs["W_sig"]), np.asarray(inputs["b_sig"])

    w0 = np.concatenate([W_ih0[_PERM].T, b0[None, :]], 0)  # [33, 2048]
    shared = {
        "w0": _bf16(w0),
        "wh0": _wT_kxn(W_hh0),
        "wi1": _wT_kxn(W_ih1),
        "wh1": _wT_kxn(W_hh1),
        "wctx": _wT_kxn(W_ihd[:, DEC_IN:DEC_IN + HID]),
        "whd": _wT_kxn(W_hhd),
        "be": _make_be(b1, bdv),
        "wcy": _bf16(np.concatenate(
            [W_ihd[_PERM][:, :DEC_IN].T, W_ihd[_PERM][:, DEC_IN + HID:].T], 0)),
        "wms": _make_wms(W_mu, W_sig, b_mu, b_sig),
    }

    in_maps = []
    for c in range(NCORES):
        sl = slice(c * BL, (c + 1) * BL)
        xe = np.ones((ENC_IN + 1, T, BL), np.float32)
        xe[:ENC_IN] = enc[sl].transpose(2, 1, 0)
        cy = np.zeros((DEC_IN + 1, HD, BL), np.float32)
        cy[:DEC_IN] = dec[sl].transpose(2, 1, 0)
        cy[DEC_IN, 1:] = tgt[sl, :HD - 1].T
        m = dict(shared)
        m["x"] = _bf16(xe)
        m["covy"] = _bf16(cy)
        in_maps.append(m)
    return in_maps


_NC_CACHE = {}


def _get_nc(T=T_ENC, HD=H_DEC):
    key = (T, HD)
    if key not in _NC_CACHE:
        _NC_CACHE[key] = build_kernel(T, HD)
    return _NC_CACHE[key]


def run(inputs, T=T_ENC, HD=H_DEC, **kw):
    nc = _get_nc(T, HD)
    in_maps = prep_inputs(inputs, T, HD)
    res = run_bass_kernel_spmd(nc, in_maps, core_ids=list(range(NCORES)), **kw)
    mu = np.concatenate([res.results[c]["mu"] for c in range(NCORES)], 0)
    sg = np.concatenate([res.results[c]["sg"] for c in range(NCORES)], 0)
    return (mu, sg), res


def kernel(**inputs):
    (mu, sg), _ = run(inputs)
    return mu, sg


# revision 15
# speedup vs baseline: 1.4462x; 1.3634x over previous
"""DeepAR (2-layer LSTM encoder + LSTM-cell decoder) Trainium2 Bass kernel.

Sharding: pure data parallel, batch 1024 -> 128 per core across 8 cores
(batch 128 == SBUF partition width).

Per-core design:
  - gates in [128 batch, 2048 gate] layout, gate order reordered to
    [g, i, f, o]: tanh(g) finishes first so the DVE chain starts early,
    and one sigmoid covers cols 512:2048.
  - all matmuls bf16 (1 cyc/col on PE), fp32 PSUM accumulation; biases ride
    ones-rows (K=1 matmuls) in bf16. (float32r is avoided: it silently
    corrupts results on HW in this setup.)
  - n-outer matmul emission: each 512-col PSUM bank completes early so
    ACT can start before the whole gate tensor is done.
  - recurrent h produced in bf16, transposed to stationary [K,M] layout by
    the DMA xbar (no PE/PSUM cost); layer-0 and layer-1 transposes go to
    different HWDGE queues (sync vs scalar) to halve queue serialization.
  - layer 1 runs one step behind layer 0 so the PE always has independent
    matmul work while layer 0's elementwise chain runs (keeps HAM warm).
  - decoder context contribution precomputed once and injected into PSUM
    via identity matmul each step; mu/sigma heads are DVE dot-products
    (scalar_tensor_tensor with accum_out), off the critical path.
"""
import numpy as np
import ml_dtypes

import concourse.bass as bass
import concourse.mybir as mybir
import concourse.tile as tile
from concourse import bacc
from concourse.bass_utils import run_bass_kernel_spmd
from concourse.masks import make_identity

F32 = mybir.dt.float32
BF16 = mybir.dt.bfloat16
AF = mybir.ActivationFunctionType
ALU = mybir.AluOpType

B, T_ENC, H_DEC = 1024, 168, 24
ENC_IN, DEC_IN, HID = 32, 16, 512
G = 4 * HID  # 2048
NCORES = 8
BL = B // NCORES  # 128 batch per core
XCHUNK = 28  # encoder-input steps per DMA chunk

# gate reorder: torch order [i, f, g, o] -> [g, i, f, o]
_PERM = np.concatenate([np.arange(1024, 1536), np.arange(0, 512),
                        np.arange(512, 1024), np.arange(1536, 2048)])


def _bf16(x):
    return np.ascontiguousarray(x.astype(ml_dtypes.bfloat16))


def _f32(x):
    return np.ascontiguousarray(x.astype(np.float32))


def _wT_kxn(W):
    """[4H, D] gate-major weight -> reordered W.T as [128, D//128, 4H] bf16."""
    Wt = W[_PERM].T  # [D, 2048]
    D = Wt.shape[0]
    return _bf16(Wt.reshape(D // 128, 128, G).transpose(1, 0, 2))


def build_kernel(T=T_ENC, HD=H_DEC):
    nc = bacc.Bacc("TRN2", target_bir_lowering=False, debug=False,
                   num_devices=NCORES)

    def din(name, shape, dt):
        return nc.dram_tensor(name, shape, dt, kind="ExternalInput").ap()

    x_d = din("x", [ENC_IN + 1, T, BL], BF16)        # enc features + ones row
    w0_d = din("w0", [ENC_IN + 1, G], BF16)           # W_ih0T + bias row
    wh0_d = din("wh0", [128, 4, G], BF16)
    wi1_d = din("wi1", [128, 4, G], BF16)
    wh1_d = din("wh1", [128, 4, G], BF16)
    wctx_d = din("wctx", [128, 4, G], BF16)
    whd_d = din("whd", [128, 4, G], BF16)
    be_d = din("be", [33, G + 128], BF16)  # row0: b1|ones, row32: bd|ones
    covy_d = din("covy", [DEC_IN + 1, HD, BL], BF16)  # dec covariates + y_prev
    wcy_d = din("wcy", [DEC_IN + 1, G], BF16)
    # head weights broadcast across partitions + per-partition biases:
    # cols 0:512 W_mu, 512:1024 W_sig, 1024 b_mu, 1025 b_sig
    wms_d = din("wms", [128, 2 * HID + 2], F32)

    mu_d = nc.dram_tensor("mu", [BL, HD], F32, kind="ExternalOutput").ap()
    sg_d = nc.dram_tensor("sg", [BL, HD], F32, kind="ExternalOutput").ap()

    with tile.TileContext(nc) as tc:
        _emit(tc, T, HD, x_d, w0_d, wh0_d, wi1_d, wh1_d, wctx_d, whd_d,
              be_d, covy_d, wcy_d, wms_d, mu_d, sg_d)
    nc.compile()
    return nc


def _emit(tc, T, HD, x_d, w0_d, wh0_d, wi1_d, wh1_d, wctx_d, whd_d,
          be_d, covy_d, wcy_d, wms_d, mu_d, sg_d):
    nc = tc.nc
    mm = nc.tensor.matmul

    with (
        tc.tile_pool(name="const", bufs=1) as cp,
        tc.tile_pool(name="xp", bufs=2) as xp,
        tc.tile_pool(name="sig", bufs=3) as sigp,
        tc.tile_pool(name="small", bufs=3) as smp,
        tc.tile_pool(name="hp", bufs=2) as hp,
        tc.tile_pool(name="htp", bufs=3) as htp,
        tc.tile_pool(name="psum", bufs=2, space="PSUM") as pp,
    ):
        # ---- persistent tiles / weight loads ----
        def load(name, dram, shape, dt):
            t = cp.tile(shape, dt, tag=name)
            nc.sync.dma_start(t[:], dram[:])
            return t

        w0 = load("w0", w0_d, [ENC_IN + 1, G], BF16)
        wh0 = load("wh0", wh0_d, [128, 4, G], BF16)
        wi1 = load("wi1", wi1_d, [128, 4, G], BF16)
        wh1 = load("wh1", wh1_d, [128, 4, G], BF16)
        wctx = load("wctx", wctx_d, [128, 4, G], BF16)
        whd = load("whd", whd_d, [128, 4, G], BF16)
        be = load("be", be_d, [33, G + 128], BF16)
        covy = load("covy", covy_d, [DEC_IN + 1, HD, BL], BF16)
        wcy = load("wcy", wcy_d, [DEC_IN + 1, G], BF16)
        wms = load("wms", wms_d, [128, 2 * HID + 2], F32)

        ident = cp.tile([128, 128], BF16, tag="ident")
        make_identity(nc, ident[:])

        ones_r = be[0:1, G:G + 128]
        ones32_r = be[32:33, G:G + 128]
        b1_r = be[0:1, 0:G]
        bd_r = be[32:33, 0:G]

        c0 = cp.tile([128, HID], F32, tag="c0")
        c1 = cp.tile([128, HID], F32, tag="c1")
        cd = cp.tile([128, HID], F32, tag="cd")
        mu_b = cp.tile([128, HD], F32, tag="mu_b")
        sp_b = cp.tile([128, HD], F32, tag="sp_b")
        sg_b = cp.tile([128, HD], F32, tag="sg_b")

        NS = G // 512  # 4 n-chunks

        def cell(g, c, first, h_tag):
            """gates psum [g|i|f|o] -> h (bf16 [128, HID]) via ACT/DVE.

            ACT order: tanh(g), sigmoid(i) first so DVE starts early.
            """
            tg = smp.tile([128, HID], F32, tag="tg")
            nc.scalar.activation(tg[:], g[:, 0:HID], AF.Tanh)
            si = smp.tile([128, HID], F32, tag="si")
            nc.scalar.activation(si[:], g[:, HID:2 * HID], AF.Sigmoid)
            sfo = sigp.tile([128, 2 * HID], F32, tag="sfo")
            nc.scalar.activation(sfo[:], g[:, 2 * HID:G], AF.Sigmoid)
            if first:
                nc.vector.tensor_mul(c[:], si[:], tg[:])
            else:
                m1 = smp.tile([128, HID], F32, tag="m1")
                nc.vector.tensor_mul(m1[:], si[:], tg[:])
                m2 = smp.tile([128, HID], F32, tag="m2")
                nc.vector.tensor_mul(m2[:], sfo[:, 0:HID], c[:])
                nc.vector.tensor_add(c[:], m1[:], m2[:])
            tcn = smp.tile([128, HID], F32, tag="tc")
            nc.scalar.activation(tcn[:], c[:], AF.Tanh)
            h = hp.tile([128, HID], BF16, tag=h_tag)
            nc.vector.tensor_mul(h[:], sfo[:, HID:2 * HID], tcn[:])
            return h

        def transp(h, tag, eng=None):
            ht = htp.tile([128, 4, 128], BF16, tag=tag)
            for k in range(4):
                e = nc.sync if k % 2 == 0 else nc.scalar
                e.dma_start(ht[:, k, :], h[:, k * 128:(k + 1) * 128],
                            transpose=True)
            return ht

        # ================= encoder =================
        # L1 runs one step behind L0: while L0(t)'s elementwise chain runs
        # on ACT/DVE/DMA, the PE stays busy on L1(t-1)'s matmuls.
        h0T_hist = {}
        h1T = None
        x_sb = None

        def layer1(t):
            nonlocal h1T
            g1 = pp.tile([128, G], F32, tag="g")
            for n in range(NS):
                s = slice(n * 512, (n + 1) * 512)
                mm(g1[:, s], ones_r, b1_r[:, s], start=True, stop=False)
            for k in range(4):
                for n in range(NS):
                    s = slice(n * 512, (n + 1) * 512)
                    mm(g1[:, s], h0T_hist[t][:, k, :], wi1[:, k, s],
                       start=False, stop=(t == 0 and k == 3))
            if t > 0:
                for k in range(4):
                    for n in range(NS):
                        s = slice(n * 512, (n + 1) * 512)
                        mm(g1[:, s], h1T[:, k, :], wh1[:, k, s],
                           start=False, stop=(k == 3))
            h1 = cell(g1, c1, t == 0, "h1")
            h1T = transp(h1, "h1T", nc.sync)

        for t in range(T):
            if t % XCHUNK == 0:
                nx = min(XCHUNK, T - t)
                x_sb = xp.tile([ENC_IN + 1, XCHUNK, BL], BF16, tag="x")
                nc.sync.dma_start(x_sb[:, :nx, :], x_d[:, t:t + nx, :])
            ti = t % XCHUNK
            # ---- layer 0, step t ----
            g0 = pp.tile([128, G], F32, tag="g")
            for n in range(NS):
                s = slice(n * 512, (n + 1) * 512)
                mm(g0[:, s], x_sb[:, ti, :], w0[:, s],
                   start=True, stop=(t == 0))
            if t > 0:
                for k in range(4):
                    for n in range(NS):
                        s = slice(n * 512, (n + 1) * 512)
                        mm(g0[:, s], h0T_hist[t - 1][:, k, :], wh0[:, k, s],
                           start=False, stop=(k == 3))
            h0 = cell(g0, c0, t == 0, "h0")
            h0T_hist[t] = transp(h0, "h0T", nc.sync)
            h0T_hist.pop(t - 2, None)
            # ---- layer 1, step t-1 ----
            if t >= 1:
                layer1(t - 1)
        layer1(T - 1)

        # ================= decoder =================
        # one-time: ctx_pre = context @ W_ctx.T + (b_ihd + b_hhd)
        cps = pp.tile([128, G], F32, tag="g")
        for n in range(NS):
            s = slice(n * 512, (n + 1) * 512)
            mm(cps[:, s], ones32_r, bd_r[:, s], start=True, stop=False)
        for k in range(4):
            for n in range(NS):
                s = slice(n * 512, (n + 1) * 512)
                mm(cps[:, s], h1T[:, k, :], wctx[:, k, s],
                   start=False, stop=(k == 3))
        ctxp = cp.tile([128, G], BF16, tag="ctxp")
        nc.scalar.copy(ctxp[:], cps[:])

        hdT = None
        for t in range(HD):
            gd = pp.tile([128, G], F32, tag="g")
            for n in range(NS):
                s = slice(n * 512, (n + 1) * 512)
                mm(gd[:, s], ident[:], ctxp[:, s], start=True, stop=False)
                mm(gd[:, s], covy[:, t, :], wcy[:, s],
                   start=False, stop=(t == 0))
            if t > 0:
                for k in range(4):
                    for n in range(NS):
                        s = slice(n * 512, (n + 1) * 512)
                        mm(gd[:, s], hdT[:, k, :], whd[:, k, s],
                           start=False, stop=(k == 3))
            hd = cell(gd, cd, t == 0, "hd")
            hdT = transp(hd, "hdT", nc.sync)

            # heads: mu/sigma dot-products on DVE, off the critical path
            hsc = smp.tile([128, HID], F32, tag="hsc")
            nc.vector.scalar_tensor_tensor(
                hsc[:], hd[:], 1.0, wms[:, 0:HID],
                op0=ALU.mult, op1=ALU.mult, accum_out=mu_b[:, t:t + 1])
            hsc2 = smp.tile([128, HID], F32, tag="hsc2")
            nc.vector.scalar_tensor_tensor(
                hsc2[:], hd[:], 1.0, wms[:, HID:2 * HID],
                op0=ALU.mult, op1=ALU.mult, accum_out=sp_b[:, t:t + 1])

        # add head biases; sigma = softplus(x) + 1e-6 via ln(exp(x)+1)
        nc.vector.tensor_scalar_add(mu_b[:], mu_b[:],
                                    wms[:, 2 * HID:2 * HID + 1])
        nc.vector.tensor_scalar_add(sp_b[:], sp_b[:],
                                    wms[:, 2 * HID + 1:2 * HID + 2])
        nc.scalar.activation(sp_b[:], sp_b[:], AF.Exp)
        nc.scalar.activation(sg_b[:], sp_b[:], AF.Ln, bias=1.0)
        nc.vector.tensor_scalar_add(sg_b[:], sg_b[:], 1e-6)
        nc.sync.dma_start(mu_d[:], mu_b[:])
        nc.sync.dma_start(sg_d[:], sg_b[:])


def _make_be(b1, bdv):
    be = np.zeros((33, G + 128), np.float32)
    be[0, :G] = b1
    be[32, :G] = bdv
    be[0, G:] = 1.0
    be[32, G:] = 1.0
    return _bf16(be)


def _make_wms(W_mu, W_sig, b_mu, b_sig):
    w = np.zeros((128, 2 * HID + 2), np.float32)
    w[:, 0:HID] = W_mu[0][None, :]
    w[:, HID:2 * HID] = W_sig[0][None, :]
    w[:, 2 * HID] = b_mu[0]
    w[:, 2 * HID + 1] = b_sig[0]
    return _f32(w)


def prep_inputs(inputs, T=T_ENC, HD=H_DEC):
    """Full-batch inputs -> list of per-core input maps (host layout prep)."""
    enc = _f32(np.asarray(inputs["enc_inp"]))[:, :T]
    dec = _f32(np.asarray(inputs["dec_inp"]))[:, :HD]
    tgt = _f32(np.asarray(inputs["tgt"]))[:, :HD]

    W_ih0, W_hh0 = np.asarray(inputs["W_ih0"]), np.asarray(inputs["W_hh0"])
    W_ih1, W_hh1 = np.asarray(inputs["W_ih1"]), np.asarray(inputs["W_hh1"])
    W_ihd, W_hhd = np.asarray(inputs["W_ihd"]), np.asarray(inputs["W_hhd"])
    b0 = _f32(np.asarray(inputs["b_ih0"]) + np.asarray(inputs["b_hh0"]))[_PERM]
    b1 = _f32(np.asarray(inputs["b_ih1"]) + np.asarray(inputs["b_hh1"]))[_PERM]
    bdv = _f32(np.asarray(inputs["b_ihd"]) + np.asarray(inputs["b_hhd"]))[_PERM]
    W_mu, b_mu = np.asarray(inputs["W_mu"]), np.asarray(inputs["b_mu"])
    W_sig, b_sig = np.asarray(inputs["W_sig"]), np.asarray(inputs["b_sig"])

    w0 = np.concatenate([W_ih0[_PERM].T, b0[None, :]], 0)  # [33, 2048]
    shared = {
        "w0": _bf16(w0),
        "wh0": _wT_kxn(W_hh0),
        "wi1": _wT_kxn(W_ih1),
        "wh1": _wT_kxn(W_hh1),
        "wctx": _wT_kxn(W_ihd[:, DEC_IN:DEC_IN + HID]),
        "whd": _wT_kxn(W_hhd),
        "be": _make_be(b1, bdv),
        "wcy": _bf16(np.concatenate(
            [W_ihd[_PERM][:, :DEC_IN].T, W_ihd[_PERM][:, DEC_IN + HID:].T], 0)),
        "wms": _make_wms(W_mu, W_sig, b_mu, b_sig),
    }

    in_maps = []
    for c in range(NCORES):
        sl = slice(c * BL, (c + 1) * BL)
        xe = np.ones((ENC_IN + 1, T, BL), np.float32)
        xe[:ENC_IN] = enc[sl].transpose(2, 1, 0)
        cy = np.zeros((DEC_IN + 1, HD, BL), np.float32)
        cy[:DEC_IN] = dec[sl].transpose(2, 1, 0)
        cy[DEC_IN, 1:] = tgt[sl, :HD - 1].T
        m = dict(shared)
        m["x"] = _bf16(xe)
        m["covy"] = _bf16(cy)
        in_maps.append(m)
    return in_maps


_NC_CACHE = {}


def _get_nc(T=T_ENC, HD=H_DEC):
    key = (T, HD)
    if key not in _NC_CACHE:
        _NC_CACHE[key] = build_kernel(T, HD)
    return _NC_CACHE[key]


def run(inputs, T=T_ENC, HD=H_DEC, **kw):
    nc = _get_nc(T, HD)
    in_maps = prep_inputs(inputs, T, HD)
    res = run_bass_kernel_spmd(nc, in_maps, core_ids=list(range(NCORES)), **kw)
    mu = np.concatenate([res.results[c]["mu"] for c in range(NCORES)], 0)
    sg = np.concatenate([res.results[c]["sg"] for c in range(NCORES)], 0)
    return (mu, sg), res


def kernel(**inputs):
    (mu, sg), _ = run(inputs)
    return mu, sg


# revision 16
# speedup vs baseline: 1.6183x; 1.1190x over previous
"""DeepAR (2-layer LSTM encoder + LSTM-cell decoder) Trainium2 Bass kernel.

Sharding: pure data parallel, batch 1024 -> 128 per core across 8 cores
(batch 128 == SBUF partition width).

Per-core design:
  - gates in [128 batch, 2048 gate] layout, gate order reordered to
    [g, i, f, o]: tanh(g) finishes first so the DVE chain starts early,
    and one sigmoid covers cols 512:2048.
  - all matmuls bf16 (1 cyc/col on PE), fp32 PSUM accumulation; biases ride
    ones-rows (K=1 matmuls) in bf16. (float32r is avoided: it silently
    corrupts results on HW in this setup.)
  - n-outer matmul emission: each 512-col PSUM bank completes early so
    ACT can start before the whole gate tensor is done.
  - recurrent h produced in bf16, transposed to stationary [K,M] layout by
    the DMA xbar (no PE/PSUM cost); layer-0 and layer-1 transposes go to
    different HWDGE queues (sync vs scalar) to halve queue serialization.
  - layer 1 runs one step behind layer 0 so the PE always has independent
    matmul work while layer 0's elementwise chain runs (keeps HAM warm).
  - decoder context contribution precomputed once and injected into PSUM
    via identity matmul each step; mu/sigma heads are DVE dot-products
    (scalar_tensor_tensor with accum_out), off the critical path.
"""
import numpy as np
import ml_dtypes

import concourse.bass as bass
import concourse.mybir as mybir
import concourse.tile as tile
from concourse import bacc
from concourse.bass_utils import run_bass_kernel_spmd
from concourse.masks import make_identity

F32 = mybir.dt.float32
BF16 = mybir.dt.bfloat16
AF = mybir.ActivationFunctionType
ALU = mybir.AluOpType

B, T_ENC, H_DEC = 1024, 168, 24
ENC_IN, DEC_IN, HID = 32, 16, 512
G = 4 * HID  # 2048
NCORES = 8
BL = B // NCORES  # 128 batch per core
XCHUNK = 28  # encoder-input steps per DMA chunk

# gate reorder: torch order [i, f, g, o] -> [g, i, f, o]
_PERM = np.concatenate([np.arange(1024, 1536), np.arange(0, 512),
                        np.arange(512, 1024), np.arange(1536, 2048)])


def _bf16(x):
    return np.ascontiguousarray(x.astype(ml_dtypes.bfloat16))


def _f32(x):
    return np.ascontiguousarray(x.astype(np.float32))


def _wT_kxn(W):
    """[4H, D] gate-major weight -> reordered W.T as [128, D//128, 4H] bf16."""
    Wt = W[_PERM].T  # [D, 2048]
    D = Wt.shape[0]
    return _bf16(Wt.reshape(D // 128, 128, G).transpose(1, 0, 2))


def build_kernel(T=T_ENC, HD=H_DEC):
    nc = bacc.Bacc("TRN2", target_bir_lowering=False, debug=False,
                   num_devices=NCORES)

    def din(name, shape, dt):
        return nc.dram_tensor(name, shape, dt, kind="ExternalInput").ap()

    x_d = din("x", [ENC_IN + 1, T, BL], BF16)        # enc features + ones row
    w0_d = din("w0", [ENC_IN + 1, G], BF16)           # W_ih0T + bias row
    wh0_d = din("wh0", [128, 4, G], BF16)
    wi1_d = din("wi1", [128, 4, G], BF16)
    wh1_d = din("wh1", [128, 4, G], BF16)
    wctx_d = din("wctx", [128, 4, G], BF16)
    whd_d = din("whd", [128, 4, G], BF16)
    be_d = din("be", [33, G + 128], BF16)  # row0: b1|ones, row32: bd|ones
    covy_d = din("covy", [DEC_IN + 1, HD, BL], BF16)  # dec covariates + y_prev
    wcy_d = din("wcy", [DEC_IN + 1, G], BF16)
    # head weights broadcast across partitions + per-partition biases:
    # cols 0:512 W_mu, 512:1024 W_sig, 1024 b_mu, 1025 b_sig
    wms_d = din("wms", [128, 2 * HID + 2], F32)

    mu_d = nc.dram_tensor("mu", [BL, HD], F32, kind="ExternalOutput").ap()
    sg_d = nc.dram_tensor("sg", [BL, HD], F32, kind="ExternalOutput").ap()

    with tile.TileContext(nc) as tc:
        _emit(tc, T, HD, x_d, w0_d, wh0_d, wi1_d, wh1_d, wctx_d, whd_d,
              be_d, covy_d, wcy_d, wms_d, mu_d, sg_d)
    nc.compile()
    return nc


def _emit(tc, T, HD, x_d, w0_d, wh0_d, wi1_d, wh1_d, wctx_d, whd_d,
          be_d, covy_d, wcy_d, wms_d, mu_d, sg_d):
    nc = tc.nc
    mm = nc.tensor.matmul

    with (
        tc.tile_pool(name="const", bufs=1) as cp,
        tc.tile_pool(name="xp", bufs=2) as xp,
        tc.tile_pool(name="sig", bufs=3) as sigp,
        tc.tile_pool(name="small", bufs=3) as smp,
        tc.tile_pool(name="hp", bufs=2) as hp,
        tc.tile_pool(name="htp", bufs=3) as htp,
        tc.tile_pool(name="psum", bufs=2, space="PSUM") as pp,
    ):
        # ---- persistent tiles / weight loads ----
        def load(name, dram, shape, dt):
            t = cp.tile(shape, dt, tag=name)
            nc.sync.dma_start(t[:], dram[:])
            return t

        w0 = load("w0", w0_d, [ENC_IN + 1, G], BF16)
        wh0 = load("wh0", wh0_d, [128, 4, G], BF16)
        wi1 = load("wi1", wi1_d, [128, 4, G], BF16)
        wh1 = load("wh1", wh1_d, [128, 4, G], BF16)
        wctx = load("wctx", wctx_d, [128, 4, G], BF16)
        whd = load("whd", whd_d, [128, 4, G], BF16)
        be = load("be", be_d, [33, G + 128], BF16)
        covy = load("covy", covy_d, [DEC_IN + 1, HD, BL], BF16)
        wcy = load("wcy", wcy_d, [DEC_IN + 1, G], BF16)
        wms = load("wms", wms_d, [128, 2 * HID + 2], F32)

        ident = cp.tile([128, 128], BF16, tag="ident")
        make_identity(nc, ident[:])

        ones_r = be[0:1, G:G + 128]
        ones32_r = be[32:33, G:G + 128]
        b1_r = be[0:1, 0:G]
        bd_r = be[32:33, 0:G]

        c0 = cp.tile([128, HID], F32, tag="c0")
        c1 = cp.tile([128, HID], F32, tag="c1")
        cd = cp.tile([128, HID], F32, tag="cd")
        mu_b = cp.tile([128, HD], F32, tag="mu_b")
        sp_b = cp.tile([128, HD], F32, tag="sp_b")
        sg_b = cp.tile([128, HD], F32, tag="sg_b")

        NS = G // 512  # 4 n-chunks

        def cell(g, c, first, h_tag):
            """gates psum [g|i|f|o] -> h (bf16 [128, HID]) via ACT/DVE.

            ACT order: tanh(g), sigmoid(i) first so DVE starts early.
            """
            tg = smp.tile([128, HID], F32, tag="tg")
            nc.scalar.activation(tg[:], g[:, 0:HID], AF.Tanh)
            si = smp.tile([128, HID], F32, tag="si")
            nc.scalar.activation(si[:], g[:, HID:2 * HID], AF.Sigmoid)
            sfo = sigp.tile([128, 2 * HID], F32, tag="sfo")
            nc.scalar.activation(sfo[:], g[:, 2 * HID:G], AF.Sigmoid)
            if first:
                nc.vector.tensor_mul(c[:], si[:], tg[:])
            else:
                m1 = smp.tile([128, HID], F32, tag="m1")
                nc.vector.tensor_mul(m1[:], si[:], tg[:])
                m2 = smp.tile([128, HID], F32, tag="m2")
                nc.vector.tensor_mul(m2[:], sfo[:, 0:HID], c[:])
                nc.vector.tensor_add(c[:], m1[:], m2[:])
            tcn = smp.tile([128, HID], F32, tag="tc")
            nc.scalar.activation(tcn[:], c[:], AF.Tanh)
            h = hp.tile([128, HID], BF16, tag=h_tag)
            nc.vector.tensor_mul(h[:], sfo[:, HID:2 * HID], tcn[:])
            return h

        def transp(h, tag, eng=None):
            ht = htp.tile([128, 4, 128], BF16, tag=tag)
            for k in range(4):
                e = nc.sync
                e.dma_start(ht[:, k, :], h[:, k * 128:(k + 1) * 128],
                            transpose=True)
            return ht

        # ================= encoder =================
        # L1 runs one step behind L0: while L0(t)'s elementwise chain runs
        # on ACT/DVE/DMA, the PE stays busy on L1(t-1)'s matmuls.
        h0T_hist = {}
        h1T = None
        x_sb = None

        def layer1(t):
            nonlocal h1T
            g1 = pp.tile([128, G], F32, tag="g")
            for n in range(NS):
                s = slice(n * 512, (n + 1) * 512)
                mm(g1[:, s], ones_r, b1_r[:, s], start=True, stop=False)
            for k in range(4):
                for n in range(NS):
                    s = slice(n * 512, (n + 1) * 512)
                    mm(g1[:, s], h0T_hist[t][:, k, :], wi1[:, k, s],
                       start=False, stop=(t == 0 and k == 3))
            if t > 0:
                for k in range(4):
                    for n in range(NS):
                        s = slice(n * 512, (n + 1) * 512)
                        mm(g1[:, s], h1T[:, k, :], wh1[:, k, s],
                           start=False, stop=(k == 3))
            h1 = cell(g1, c1, t == 0, "h1")
            h1T = transp(h1, "h1T", nc.sync)

        for t in range(T):
            if t % XCHUNK == 0:
                nx = min(XCHUNK, T - t)
                x_sb = xp.tile([ENC_IN + 1, XCHUNK, BL], BF16, tag="x")
                nc.sync.dma_start(x_sb[:, :nx, :], x_d[:, t:t + nx, :])
            ti = t % XCHUNK
            # ---- layer 0, step t ----
            g0 = pp.tile([128, G], F32, tag="g")
            for n in range(NS):
                s = slice(n * 512, (n + 1) * 512)
                mm(g0[:, s], x_sb[:, ti, :], w0[:, s],
                   start=True, stop=(t == 0))
            if t > 0:
                for k in range(4):
                    for n in range(NS):
                        s = slice(n * 512, (n + 1) * 512)
                        mm(g0[:, s], h0T_hist[t - 1][:, k, :], wh0[:, k, s],
                           start=False, stop=(k == 3))
            h0 = cell(g0, c0, t == 0, "h0")
            h0T_hist[t] = transp(h0, "h0T", nc.sync)
            h0T_hist.pop(t - 2, None)
            # ---- layer 1, step t-1 ----
            if t >= 1:
                layer1(t - 1)
        layer1(T - 1)

        # ================= decoder =================
        # one-time: ctx_pre = context @ W_ctx.T + (b_ihd + b_hhd)
        cps = pp.tile([128, G], F32, tag="g")
        for n in range(NS):
            s = slice(n * 512, (n + 1) * 512)
            mm(cps[:, s], ones32_r, bd_r[:, s], start=True, stop=False)
        for k in range(4):
            for n in range(NS):
                s = slice(n * 512, (n + 1) * 512)
                mm(cps[:, s], h1T[:, k, :], wctx[:, k, s],
                   start=False, stop=(k == 3))
        ctxp = cp.tile([128, G], BF16, tag="ctxp")
        nc.scalar.copy(ctxp[:], cps[:])

        hdT = None
        for t in range(HD):
            gd = pp.tile([128, G], F32, tag="g")
            for n in range(NS):
                s = slice(n * 512, (n + 1) * 512)
                mm(gd[:, s], ident[:], ctxp[:, s], start=True, stop=False)
                mm(gd[:, s], covy[:, t, :], wcy[:, s],
                   start=False, stop=(t == 0))
            if t > 0:
                for k in range(4):
                    for n in range(NS):
                        s = slice(n * 512, (n + 1) * 512)
                        mm(gd[:, s], hdT[:, k, :], whd[:, k, s],
                           start=False, stop=(k == 3))
            hd = cell(gd, cd, t == 0, "hd")
            hdT = transp(hd, "hdT", nc.sync)

            # heads: mu/sigma dot-products on DVE, off the critical path
            hsc = smp.tile([128, HID], F32, tag="hsc")
            nc.vector.scalar_tensor_tensor(
                hsc[:], hd[:], 1.0, wms[:, 0:HID],
                op0=ALU.mult, op1=ALU.mult, accum_out=mu_b[:, t:t + 1])
            hsc2 = smp.tile([128, HID], F32, tag="hsc2")
            nc.vector.scalar_tensor_tensor(
                hsc2[:], hd[:], 1.0, wms[:, HID:2 * HID],
                op0=ALU.mult, op1=ALU.mult, accum_out=sp_b[:, t:t + 1])

        # add head biases; sigma = softplus(x) + 1e-6 via ln(exp(x)+1)
        nc.vector.tensor_scalar_add(mu_b[:], mu_b[:],
                                    wms[:, 2 * HID:2 * HID + 1])
        nc.vector.tensor_scalar_add(sp_b[:], sp_b[:],
                                    wms[:, 2 * HID + 1:2 * HID + 2])
        nc.scalar.activation(sp_b[:], sp_b[:], AF.Exp)
        nc.scalar.activation(sg_b[:], sp_b[:], AF.Ln, bias=1.0)
        nc.vector.tensor_scalar_add(sg_b[:], sg_b[:], 1e-6)
        nc.sync.dma_start(mu_d[:], mu_b[:])
        nc.sync.dma_start(sg_d[:], sg_b[:])


def _make_be(b1, bdv):
    be = np.zeros((33, G + 128), np.float32)
    be[0, :G] = b1
    be[32, :G] = bdv
    be[0, G:] = 1.0
    be[32, G:] = 1.0
    return _bf16(be)


def _make_wms(W_mu, W_sig, b_mu, b_sig):
    w = np.zeros((128, 2 * HID + 2), np.float32)
    w[:, 0:HID] = W_mu[0][None, :]
    w[:, HID:2 * HID] = W_sig[0][None, :]
    w[:, 2 * HID] = b_mu[0]
    w[:, 2 * HID + 1] = b_sig[0]
    return _f32(w)


def prep_inputs(inputs, T=T_ENC, HD=H_DEC):
    """Full-batch inputs -> list of per-core input maps (host layout prep)."""
    enc = _f32(np.asarray(inputs["enc_inp"]))[:, :T]
    dec = _f32(np.asarray(inputs["dec_inp"]))[:, :HD]
    tgt = _f32(np.asarray(inputs["tgt"]))[:, :HD]

    W_ih0, W_hh0 = np.asarray(inputs["W_ih0"]), np.asarray(inputs["W_hh0"])
    W_ih1, W_hh1 = np.asarray(inputs["W_ih1"]), np.asarray(inputs["W_hh1"])
    W_ihd, W_hhd = np.asarray(inputs["W_ihd"]), np.asarray(inputs["W_hhd"])
    b0 = _f32(np.asarray(inputs["b_ih0"]) + np.asarray(inputs["b_hh0"]))[_PERM]
    b1 = _f32(np.asarray(inputs["b_ih1"]) + np.asarray(inputs["b_hh1"]))[_PERM]
    bdv = _f32(np.asarray(inputs["b_ihd"]) + np.asarray(inputs["b_hhd"]))[_PERM]
    W_mu, b_mu = np.asarray(inputs["W_mu"]), np.asarray(inputs["b_mu"])
    W_sig, b_sig = np.asarray(inputs["W_sig"]), np.asarray(inputs["b_sig"])

    w0 = np.concatenate([W_ih0[_PERM].T, b0[None, :]], 0)  # [33, 2048]
    shared = {
        "w0": _bf16(w0),
        "wh0": _wT_kxn(W_hh0),
        "wi1": _wT_kxn(W_ih1),
        "wh1": _wT_kxn(W_hh1),
        "wctx": _wT_kxn(W_ihd[:, DEC_IN:DEC_IN + HID]),
        "whd": _wT_kxn(W_hhd),
        "be": _make_be(b1, bdv),
        "wcy": _bf16(np.concatenate(
            [W_ihd[_PERM][:, :DEC_IN].T, W_ihd[_PERM][:, DEC_IN + HID:].T], 0)),
        "wms": _make_wms(W_mu, W_sig, b_mu, b_sig),
    }

    in_maps = []
    for c in range(NCORES):
        sl = slice(c * BL, (c + 1) * BL)
        xe = np.ones((ENC_IN + 1, T, BL), np.float32)
        xe[:ENC_IN] = enc[sl].transpose(2, 1, 0)
        cy = np.zeros((DEC_IN + 1, HD, BL), np.float32)
        cy[:DEC_IN] = dec[sl].transpose(2, 1, 0)
        cy[DEC_IN, 1:] = tgt[sl, :HD - 1].T
        m = dict(shared)
        m["x"] = _bf16(xe)
        m["covy"] = _bf16(cy)
        in_maps.append(m)
    return in_maps


_NC_CACHE = {}


def _get_nc(T=T_ENC, HD=H_DEC):
    key = (T, HD)
    if key not in _NC_CACHE:
        _NC_CACHE[key] = build_kernel(T, HD)
    return _NC_CACHE[key]


def run(inputs, T=T_ENC, HD=H_DEC, **kw):
    nc = _get_nc(T, HD)
    in_maps = prep_inputs(inputs, T, HD)
    res = run_bass_kernel_spmd(nc, in_maps, core_ids=list(range(NCORES)), **kw)
    mu = np.concatenate([res.results[c]["mu"] for c in range(NCORES)], 0)
    sg = np.concatenate([res.results[c]["sg"] for c in range(NCORES)], 0)
    return (mu, sg), res


def kernel(**inputs):
    (mu, sg), _ = run(inputs)
    return mu, sg


# revision 17
# speedup vs baseline: 1.6303x; 1.0074x over previous
"""DeepAR (2-layer LSTM encoder + LSTM-cell decoder) Trainium2 Bass kernel.

Sharding: pure data parallel, batch 1024 -> 128 per core across 8 cores
(batch 128 == SBUF partition width).

Per-core design:
  - gates in [128 batch, 2048 gate] layout, gate order reordered to
    [g, i, f, o]: tanh(g) finishes first so the DVE chain starts early,
    and one sigmoid covers cols 512:2048.
  - all matmuls bf16 (1 cyc/col on PE), fp32 PSUM accumulation; biases ride
    ones-rows (K=1 matmuls) in bf16. (float32r is avoided: it silently
    corrupts results on HW in this setup.)
  - n-outer matmul emission: each 512-col PSUM bank completes early so
    ACT can start before the whole gate tensor is done.
  - recurrent h produced in bf16, transposed to stationary [K,M] layout by
    the DMA xbar (no PE/PSUM cost); layer-0 and layer-1 transposes go to
    different HWDGE queues (sync vs scalar) to halve queue serialization.
  - layer 1 runs one step behind layer 0 so the PE always has independent
    matmul work while layer 0's elementwise chain runs (keeps HAM warm).
  - decoder context contribution precomputed once and injected into PSUM
    via identity matmul each step; mu/sigma heads are DVE dot-products
    (scalar_tensor_tensor with accum_out), off the critical path.
"""
import numpy as np
import ml_dtypes

import concourse.bass as bass
import concourse.mybir as mybir
import concourse.tile as tile
from concourse import bacc
from concourse.bass_utils import run_bass_kernel_spmd
from concourse.masks import make_identity

F32 = mybir.dt.float32
BF16 = mybir.dt.bfloat16
AF = mybir.ActivationFunctionType
ALU = mybir.AluOpType

B, T_ENC, H_DEC = 1024, 168, 24
ENC_IN, DEC_IN, HID = 32, 16, 512
G = 4 * HID  # 2048
NCORES = 8
BL = B // NCORES  # 128 batch per core
XCHUNK = 28  # encoder-input steps per DMA chunk

# gate reorder: torch order [i, f, g, o] -> [g, i, f, o]
_PERM = np.concatenate([np.arange(1024, 1536), np.arange(0, 512),
                        np.arange(512, 1024), np.arange(1536, 2048)])


def _bf16(x):
    return np.ascontiguousarray(x.astype(ml_dtypes.bfloat16))


def _f32(x):
    return np.ascontiguousarray(x.astype(np.float32))


def _wT_kxn(W):
    """[4H, D] gate-major weight -> reordered W.T as [128, D//128, 4H] bf16."""
    Wt = W[_PERM].T  # [D, 2048]
    D = Wt.shape[0]
    return _bf16(Wt.reshape(D // 128, 128, G).transpose(1, 0, 2))


def build_kernel(T=T_ENC, HD=H_DEC):
    nc = bacc.Bacc("TRN2", target_bir_lowering=False, debug=False,
                   num_devices=NCORES)

    def din(name, shape, dt):
        return nc.dram_tensor(name, shape, dt, kind="ExternalInput").ap()

    x_d = din("x", [ENC_IN + 1, T, BL], BF16)        # enc features + ones row
    w0_d = din("w0", [ENC_IN + 1, G], BF16)           # W_ih0T + bias row
    wh0_d = din("wh0", [128, 4, G], BF16)
    wi1_d = din("wi1", [128, 4, G], BF16)
    wh1_d = din("wh1", [128, 4, G], BF16)
    wctx_d = din("wctx", [128, 4, G], BF16)
    whd_d = din("whd", [128, 4, G], BF16)
    be_d = din("be", [33, G + 128], BF16)  # row0: b1|ones, row32: bd|ones
    covy_d = din("covy", [DEC_IN + 1, HD, BL], BF16)  # dec covariates + y_prev
    wcy_d = din("wcy", [DEC_IN + 1, G], BF16)
    # head weights broadcast across partitions + per-partition biases:
    # cols 0:512 W_mu, 512:1024 W_sig, 1024 b_mu, 1025 b_sig
    wms_d = din("wms", [128, 2 * HID + 2], F32)

    mu_d = nc.dram_tensor("mu", [BL, HD], F32, kind="ExternalOutput").ap()
    sg_d = nc.dram_tensor("sg", [BL, HD], F32, kind="ExternalOutput").ap()

    with tile.TileContext(nc) as tc:
        _emit(tc, T, HD, x_d, w0_d, wh0_d, wi1_d, wh1_d, wctx_d, whd_d,
              be_d, covy_d, wcy_d, wms_d, mu_d, sg_d)
    nc.compile()
    return nc


def _emit(tc, T, HD, x_d, w0_d, wh0_d, wi1_d, wh1_d, wctx_d, whd_d,
          be_d, covy_d, wcy_d, wms_d, mu_d, sg_d):
    nc = tc.nc
    mm = nc.tensor.matmul

    with (
        tc.tile_pool(name="const", bufs=1) as cp,
        tc.tile_pool(name="xp", bufs=2) as xp,
        tc.tile_pool(name="sig", bufs=3) as sigp,
        tc.tile_pool(name="small", bufs=3) as smp,
        tc.tile_pool(name="hp", bufs=2) as hp,
        tc.tile_pool(name="htp", bufs=3) as htp,
        tc.tile_pool(name="psum", bufs=2, space="PSUM") as pp,
    ):
        # ---- persistent tiles / weight loads ----
        def load(name, dram, shape, dt):
            t = cp.tile(shape, dt, tag=name)
            nc.sync.dma_start(t[:], dram[:])
            return t

        w0 = load("w0", w0_d, [ENC_IN + 1, G], BF16)
        wh0 = load("wh0", wh0_d, [128, 4, G], BF16)
        wi1 = load("wi1", wi1_d, [128, 4, G], BF16)
        wh1 = load("wh1", wh1_d, [128, 4, G], BF16)
        wctx = load("wctx", wctx_d, [128, 4, G], BF16)
        whd = load("whd", whd_d, [128, 4, G], BF16)
        be = load("be", be_d, [33, G + 128], BF16)
        covy = load("covy", covy_d, [DEC_IN + 1, HD, BL], BF16)
        wcy = load("wcy", wcy_d, [DEC_IN + 1, G], BF16)
        wms = load("wms", wms_d, [128, 2 * HID + 2], F32)

        ident = cp.tile([128, 128], BF16, tag="ident")
        make_identity(nc, ident[:])

        ones_r = be[0:1, G:G + 128]
        ones32_r = be[32:33, G:G + 128]
        b1_r = be[0:1, 0:G]
        bd_r = be[32:33, 0:G]

        c0 = cp.tile([128, HID], F32, tag="c0")
        c1 = cp.tile([128, HID], F32, tag="c1")
        cd = cp.tile([128, HID], F32, tag="cd")
        mu_b = cp.tile([128, HD], F32, tag="mu_b")
        sp_b = cp.tile([128, HD], F32, tag="sp_b")
        sg_b = cp.tile([128, HD], F32, tag="sg_b")

        NS = G // 512  # 4 n-chunks

        def cell(g, c, first, h_tag):
            """gates psum [g|i|f|o] -> h (bf16 [128, HID]) via ACT/DVE.

            ACT order: tanh(g), sigmoid(i) first so DVE starts early.
            """
            tg = smp.tile([128, HID], F32, tag="tg")
            nc.scalar.activation(tg[:], g[:, 0:HID], AF.Tanh)
            si = smp.tile([128, HID], F32, tag="si")
            nc.scalar.activation(si[:], g[:, HID:2 * HID], AF.Sigmoid)
            sfo = sigp.tile([128, 2 * HID], F32, tag="sfo")
            nc.scalar.activation(sfo[:], g[:, 2 * HID:G], AF.Sigmoid)
            if first:
                nc.vector.tensor_mul(c[:], si[:], tg[:])
            else:
                m1 = smp.tile([128, HID], F32, tag="m1")
                nc.vector.tensor_mul(m1[:], si[:], tg[:])
                m2 = smp.tile([128, HID], F32, tag="m2")
                nc.vector.tensor_mul(m2[:], sfo[:, 0:HID], c[:])
                nc.vector.tensor_add(c[:], m1[:], m2[:])
            tcn = smp.tile([128, HID], F32, tag="tc")
            nc.scalar.activation(tcn[:], c[:], AF.Tanh)
            h = hp.tile([128, HID], BF16, tag=h_tag)
            hh = HID // 2
            nc.vector.tensor_mul(h[:, 0:hh], sfo[:, HID:HID + hh], tcn[:, 0:hh])
            nc.vector.tensor_mul(h[:, hh:HID], sfo[:, HID + hh:2 * HID],
                                 tcn[:, hh:HID])
            return h

        def transp(h, tag, split=False):
            ht = htp.tile([128, 4, 128], BF16, tag=tag)
            for k in range(4):
                e = nc.scalar if (split and k % 2 == 1) else nc.sync
                e.dma_start(ht[:, k, :], h[:, k * 128:(k + 1) * 128],
                            transpose=True)
            return ht

        # ================= encoder =================
        # L1 runs one step behind L0: while L0(t)'s elementwise chain runs
        # on ACT/DVE/DMA, the PE stays busy on L1(t-1)'s matmuls.
        h0T_hist = {}
        h1T = None
        x_sb = None

        def layer1(t):
            nonlocal h1T
            g1 = pp.tile([128, G], F32, tag="g")
            for n in range(NS):
                s = slice(n * 512, (n + 1) * 512)
                mm(g1[:, s], ones_r, b1_r[:, s], start=True, stop=False)
            for k in range(4):
                for n in range(NS):
                    s = slice(n * 512, (n + 1) * 512)
                    mm(g1[:, s], h0T_hist[t][:, k, :], wi1[:, k, s],
                       start=False, stop=(t == 0 and k == 3))
            if t > 0:
                for k in range(4):
                    for n in range(NS):
                        s = slice(n * 512, (n + 1) * 512)
                        mm(g1[:, s], h1T[:, k, :], wh1[:, k, s],
                           start=False, stop=(k == 3))
            h1 = cell(g1, c1, t == 0, "h1")
            h1T = transp(h1, "h1T")

        for t in range(T):
            if t % XCHUNK == 0:
                nx = min(XCHUNK, T - t)
                x_sb = xp.tile([ENC_IN + 1, XCHUNK, BL], BF16, tag="x")
                nc.sync.dma_start(x_sb[:, :nx, :], x_d[:, t:t + nx, :])
            ti = t % XCHUNK
            # ---- layer 0, step t ----
            g0 = pp.tile([128, G], F32, tag="g")
            for n in range(NS):
                s = slice(n * 512, (n + 1) * 512)
                mm(g0[:, s], x_sb[:, ti, :], w0[:, s],
                   start=True, stop=(t == 0))
            if t > 0:
                for k in range(4):
                    for n in range(NS):
                        s = slice(n * 512, (n + 1) * 512)
                        mm(g0[:, s], h0T_hist[t - 1][:, k, :], wh0[:, k, s],
                           start=False, stop=(k == 3))
            h0 = cell(g0, c0, t == 0, "h0")
            h0T_hist[t] = transp(h0, "h0T")
            h0T_hist.pop(t - 2, None)
            # ---- layer 1, step t-1 ----
            if t >= 1:
                layer1(t - 1)
        layer1(T - 1)

        # ================= decoder =================
        # one-time: ctx_pre = context @ W_ctx.T + (b_ihd + b_hhd)
        cps = pp.tile([128, G], F32, tag="g")
        for n in range(NS):
            s = slice(n * 512, (n + 1) * 512)
            mm(cps[:, s], ones32_r, bd_r[:, s], start=True, stop=False)
        for k in range(4):
            for n in range(NS):
                s = slice(n * 512, (n + 1) * 512)
                mm(cps[:, s], h1T[:, k, :], wctx[:, k, s],
                   start=False, stop=(k == 3))
        ctxp = cp.tile([128, G], BF16, tag="ctxp")
        nc.scalar.copy(ctxp[:], cps[:])

        hdT = None
        for t in range(HD):
            gd = pp.tile([128, G], F32, tag="g")
            for n in range(NS):
                s = slice(n * 512, (n + 1) * 512)
                mm(gd[:, s], ident[:], ctxp[:, s], start=True, stop=False)
                mm(gd[:, s], covy[:, t, :], wcy[:, s],
                   start=False, stop=(t == 0))
            if t > 0:
                for k in range(4):
                    for n in range(NS):
                        s = slice(n * 512, (n + 1) * 512)
                        mm(gd[:, s], hdT[:, k, :], whd[:, k, s],
                           start=False, stop=(k == 3))
            hd = cell(gd, cd, t == 0, "hd")
            hdT = transp(hd, "hdT", split=True)

            # heads: mu/sigma dot-products on DVE, off the critical path
            hsc = smp.tile([128, HID], F32, tag="hsc")
            nc.vector.scalar_tensor_tensor(
                hsc[:], hd[:], 1.0, wms[:, 0:HID],
                op0=ALU.mult, op1=ALU.mult, accum_out=mu_b[:, t:t + 1])
            hsc2 = smp.tile([128, HID], F32, tag="hsc2")
            nc.vector.scalar_tensor_tensor(
                hsc2[:], hd[:], 1.0, wms[:, HID:2 * HID],
                op0=ALU.mult, op1=ALU.mult, accum_out=sp_b[:, t:t + 1])

        # add head biases; sigma = softplus(x) + 1e-6 via ln(exp(x)+1)
        nc.vector.tensor_scalar_add(mu_b[:], mu_b[:],
                                    wms[:, 2 * HID:2 * HID + 1])
        nc.vector.tensor_scalar_add(sp_b[:], sp_b[:],
                                    wms[:, 2 * HID + 1:2 * HID + 2])
        nc.scalar.activation(sp_b[:], sp_b[:], AF.Exp)
        nc.scalar.activation(sg_b[:], sp_b[:], AF.Ln, bias=1.0)
        nc.vector.tensor_scalar_add(sg_b[:], sg_b[:], 1e-6)
        nc.sync.dma_start(mu_d[:], mu_b[:])
        nc.sync.dma_start(sg_d[:], sg_b[:])


def _make_be(b1, bdv):
    be = np.zeros((33, G + 128), np.float32)
    be[0, :G] = b1
    be[32, :G] = bdv
    be[0, G:] = 1.0
    be[32, G:] = 1.0
    return _bf16(be)


def _make_wms(W_mu, W_sig, b_mu, b_sig):
    w = np.zeros((128, 2 * HID + 2), np.float32)
    w[:, 0:HID] = W_mu[0][None, :]
    w[:, HID:2 * HID] = W_sig[0][None, :]
    w[:, 2 * HID] = b_mu[0]
    w[:, 2 * HID + 1] = b_sig[0]
    return _f32(w)


def prep_inputs(inputs, T=T_ENC, HD=H_DEC):
    """Full-batch inputs -> list of per-core input maps (host layout prep)."""
    enc = _f32(np.asarray(inputs["enc_inp"]))[:, :T]
    dec = _f32(np.asarray(inputs["dec_inp"]))[:, :HD]
    tgt = _f32(np.asarray(inputs["tgt"]))[:, :HD]

    W_ih0, W_hh0 = np.asarray(inputs["W_ih0"]), np.asarray(inputs["W_hh0"])
    W_ih1, W_hh1 = np.asarray(inputs["W_ih1"]), np.asarray(inputs["W_hh1"])
    W_ihd, W_hhd = np.asarray(inputs["W_ihd"]), np.asarray(inputs["W_hhd"])
    b0 = _f32(np.asarray(inputs["b_ih0"]) + np.asarray(inputs["b_hh0"]))[_PERM]
    b1 = _f32(np.asarray(inputs["b_ih1"]) + np.asarray(inputs["b_hh1"]))[_PERM]
    bdv = _f32(np.asarray(inputs["b_ihd"]) + np.asarray(inputs["b_hhd"]))[_PERM]
    W_mu, b_mu = np.asarray(inputs["W_mu"]), np.asarray(inputs["b_mu"])
    W_sig, b_sig = np.asarray(inputs["W_sig"]), np.asarray(inputs["b_sig"])

    w0 = np.concatenate([W_ih0[_PERM].T, b0[None, :]], 0)  # [33, 2048]
    shared = {
        "w0": _bf16(w0),
        "wh0": _wT_kxn(W_hh0),
        "wi1": _wT_kxn(W_ih1),
        "wh1": _wT_kxn(W_hh1),
        "wctx": _wT_kxn(W_ihd[:, DEC_IN:DEC_IN + HID]),
        "whd": _wT_kxn(W_hhd),
        "be": _make_be(b1, bdv),
        "wcy": _bf16(np.concatenate(
            [W_ihd[_PERM][:, :DEC_IN].T, W_ihd[_PERM][:, DEC_IN + HID:].T], 0)),
        "wms": _make_wms(W_mu, W_sig, b_mu, b_sig),
    }

    in_maps = []
    for c in range(NCORES):
        sl = slice(c * BL, (c + 1) * BL)
        xe = np.ones((ENC_IN + 1, T, BL), np.float32)
        xe[:ENC_IN] = enc[sl].transpose(2, 1, 0)
        cy = np.zeros((DEC_IN + 1, HD, BL), np.float32)
        cy[:DEC_IN] = dec[sl].transpose(2, 1, 0)
        cy[DEC_IN, 1:] = tgt[sl, :HD - 1].T
        m = dict(shared)
        m["x"] = _bf16(xe)
        m["covy"] = _bf16(cy)
        in_maps.append(m)
    return in_maps


_NC_CACHE = {}


def _get_nc(T=T_ENC, HD=H_DEC):
    key = (T, HD)
    if key not in _NC_CACHE:
        _NC_CACHE[key] = build_kernel(T, HD)
    return _NC_CACHE[key]


def run(inputs, T=T_ENC, HD=H_DEC, **kw):
    nc = _get_nc(T, HD)
    in_maps = prep_inputs(inputs, T, HD)
    res = run_bass_kernel_spmd(nc, in_maps, core_ids=list(range(NCORES)), **kw)
    mu = np.concatenate([res.results[c]["mu"] for c in range(NCORES)], 0)
    sg = np.concatenate([res.results[c]["sg"] for c in range(NCORES)], 0)
    return (mu, sg), res


def kernel(**inputs):
    (mu, sg), _ = run(inputs)
    return mu, sg
